# revision 27
# baseline (speedup 1.0000x reference)
"""Bahdanau inter-attention Trainium2 kernel (data-parallel over batch, 8 cores).

Per-core computation (BL=8 batches):
  enc_proj = enc @ W_enc.T          (bf16 PE matmul, H contracted)
  energy   = tanh(enc_proj + dec_proj)   (ACT, per-partition bias)
  scores   = energy @ v              (bf16 PE matmul, A contracted)
  expw     = exp(scores)             (ACT; no max-subtraction: |scores| < ~13)
  context  = (expw @ enc) / sum(expw)
  attn     = expw / sum(expw)

enc is loaded once from HBM with an fp32->bf16 cast in the DMA (SWDGE) and kept
in natural [T_p, H] layout for the context matmul. The projection matmul needs
H on partitions; the on-chip transpose runs (mostly) on the PE in "fp32-pair"
mode: two adjacent bf16 H-channels are moved as one fp32 element, halving the
LDWEIGHTS-bound transpose cost. The projection matmul then reads the pair
layout with stride-2 bf16 access patterns (full rate, measured). A tunable
number of segments instead use the XBAR DMA transpose (standard layout) to
offload the PE.
"""

import numpy as np

import concourse.bass as bass
import concourse.tile as tile
from concourse import bacc, mybir
from concourse.bass_utils import run_bass_kernel_spmd
from concourse.masks import make_identity

F32 = mybir.dt.float32
BF16 = mybir.dt.bfloat16

B, T, H, A = 64, 4096, 512, 256
NCORES = 8
BL = B // NCORES          # 8 batches per core
P = 128

# ---- tunables ----
T_SEG = 2048              # T tile per DMA load
TC = 512                  # matmul moving-dim chunk
N_XBAR_SEG = 0            # of the 16 (b, seg) units, how many transpose via XBAR

ACH = A // P              # A chunks of 128
HS = H // P               # H subtiles (contraction)


def build_kernel(n_xbar_seg=N_XBAR_SEG, t_seg=T_SEG):
    nseg = T // t_seg
    ntc = t_seg // TC
    no = t_seg // P
    no_per_tc = TC // P
    gtot = BL * nseg
    if n_xbar_seg > 0:
        xbar_set = set(int(round(i * gtot / n_xbar_seg)) % gtot
                       for i in range(n_xbar_seg))
    else:
        xbar_set = set()

    nc = bacc.Bacc("TRN2", target_bir_lowering=False, debug=False)

    enc = nc.dram_tensor("enc", [BL, T, H], F32, kind="ExternalInput")
    dec = nc.dram_tensor("dec", [BL, H], F32, kind="ExternalInput")
    w_enc = nc.dram_tensor("w_enc", [A, H], F32, kind="ExternalInput")
    w_dec = nc.dram_tensor("w_dec", [A, H], F32, kind="ExternalInput")
    v_in = nc.dram_tensor("v", [1, A], F32, kind="ExternalInput")
    ctx_out = nc.dram_tensor("ctx", [BL, H], F32, kind="ExternalOutput")
    attn_out = nc.dram_tensor("attn", [BL, T], F32, kind="ExternalOutput")

    with tile.TileContext(nc) as tc:
        with tc.tile_pool(name="singles", bufs=1) as singles, \
             tc.tile_pool(name="enc_nat", bufs=3) as enc_nat_pool, \
             tc.tile_pool(name="enc_t", bufs=2) as enc_t_pool, \
             tc.tile_pool(name="energy", bufs=4) as energy_pool, \
             tc.tile_pool(name="small", bufs=3) as small_pool, \
             tc.tile_pool(name="epil", bufs=1) as epil_pool, \
             tc.tile_pool(name="dram", bufs=1, space="DRAM") as dram_pool, \
             tc.tile_pool(name="ps_proj", bufs=3, space="PSUM") as ps_proj, \
             tc.tile_pool(name="ps_sc", bufs=1, space="PSUM") as ps_sc, \
             tc.tile_pool(name="ps_tiny", bufs=1, space="PSUM") as ps_tiny, \
             tc.tile_pool(name="ps_petr", bufs=2, space="PSUM") as ps_petr, \
             tc.tile_pool(name="ps_ctx", bufs=1, space="PSUM") as ps_ctx:

            # ---------------- prep: identities ----------------
            ident_bf = singles.tile([P, P], BF16)
            make_identity(nc, ident_bf[:])
            identf = singles.tile([P, P], F32)
            make_identity(nc, identf[:])
            ident1 = singles.tile([1, 1], F32)
            nc.vector.memset(ident1[:], 1.0)

            # -------- prep: weights --------
            # standard transposed [H_p, hs, A] bf16 (for XBAR segs)
            # pair layout [j, u, ac, a] fp32-pairs: elem = (W[a, 256u+2j], W[a, 256u+2j+1])
            def load_w(w_dram, name, want_std):
                w_sb = small_pool.tile([P, ACH, H], BF16, tag="w_load")
                nc.gpsimd.dma_start(
                    w_sb[:], w_dram.rearrange("(s p) h -> p s h", p=P))
                wT = None
                if want_std:
                    wT = singles.tile([P, HS, A], BF16, tag=f"wT_{name}")
                    for asub in range(ACH):
                        for hs in range(HS):
                            pst = ps_proj.tile([P, P], BF16, tag="proj")
                            nc.tensor.transpose(
                                pst[:], w_sb[:, asub, hs * P:(hs + 1) * P],
                                ident_bf[:])
                            nc.vector.tensor_copy(
                                wT[:, hs, asub * P:(asub + 1) * P], pst[:])
                w2 = singles.tile([P, 2, ACH, P], F32, tag=f"w2_{name}")
                w_f32 = w_sb.bitcast(F32)  # [P, ACH, 256]
                for u in range(2):
                    for asub in range(ACH):
                        pst = ps_proj.tile([P, P], F32, tag="proj")
                        nc.tensor.transpose(
                            pst[:], w_f32[:, asub, u * P:(u + 1) * P], identf[:])
                        nc.vector.tensor_copy(w2[:, u, asub, :], pst[:])
                return wT, w2.bitcast(BF16)  # w2 view [P, 2, ACH, 2P] (a,parity interleaved)

            w_encT, w2e = load_w(w_enc, "enc", n_xbar_seg > 0)
            _, w2d = load_w(w_dec, "dec", False)

            # v as [A_p, ACH] bf16: v_sb[p, c] = v[0, c*128+p]
            v_sb = singles.tile([P, ACH], BF16)
            nc.gpsimd.dma_start(v_sb[:], v_in[0].rearrange("(c p) -> p c", p=P))

            # -------- prep: dec_proj [A_p, ACH, BL] fp32 --------
            dec_sb = small_pool.tile([BL, H], BF16, tag="dec_load")
            nc.gpsimd.dma_start(dec_sb[:], dec[:])
            dec_f32 = dec_sb.bitcast(F32)  # [BL, 256]
            decT2 = singles.tile([P, 2, BL], F32)
            for u in range(2):
                pst = ps_proj.tile([P, BL], F32, tag="proj")
                nc.tensor.transpose(
                    pst[:], dec_f32[:, u * P:(u + 1) * P], identf[:BL, :BL])
                nc.vector.tensor_copy(decT2[:, u, :], pst[:])
            decT2v = decT2.bitcast(BF16)  # [P, 2, 2*BL]
            dec_proj = singles.tile([P, ACH, BL], F32)
            for ac in range(ACH):
                psd = ps_proj.tile([P, BL], F32, tag="proj")
                for k, (u, par) in enumerate([(0, 0), (0, 1), (1, 0), (1, 1)]):
                    nc.tensor.matmul(
                        psd[:], w2d[:, u, ac, par::2], decT2v[:, u, par::2],
                        start=(k == 0), stop=(k == 3))
                nc.vector.tensor_copy(dec_proj[:, ac, :], psd[:])

            # DRAM scratch for unnormalized per-batch rows (engine APs cannot
            # start at partition b; DMA can address any DRAM row).
            attn_scr = dram_pool.tile([BL, T], F32)
            ctx_scr4 = dram_pool.tile([BL, 4, H], F32)
            sums_all = singles.tile([1, BL], F32)

            # ---------------- main loop ----------------
            for b in range(BL):
                ctx_ps = ps_ctx.tile([P, H], F32, tag="ctx")
                sumparts = small_pool.tile([1, nseg * ntc], F32, tag="sumparts")
                for seg in range(nseg):
                    g = b * nseg + seg
                    use_xbar = g in xbar_set
                    t0 = seg * t_seg
                    # load natural tile [128, no, H] bf16 (cast in DMA)
                    e_nat = enc_nat_pool.tile([P, no, H], BF16, tag="enat")
                    nc.gpsimd.dma_start(
                        e_nat[:],
                        enc[b, t0:t0 + t_seg, :].rearrange("(o p) h -> p o h", p=P))

                    if use_xbar:
                        # one batched XBAR transpose -> [H_p, o, hs, 128] bf16,
                        # alternating HWDGE rings (SP / ACT)
                        e_tx = enc_t_pool.tile([P, no, HS, P], BF16, tag="etr_x")
                        ring = nc.sync if g % 2 == 0 else nc.scalar
                        ring.dma_start(e_tx[:], e_nat[:], transpose=True)
                    else:
                        # PE fp32-pair transposes -> [j, o, u, t] fp32-pairs
                        e_tp = enc_t_pool.tile([P, no, 2, P], F32, tag="etr_p")
                        e_nf = e_nat.bitcast(F32)  # [P, no, 256]
                        for op in range(0, no, 2):
                            pst = ps_petr.tile([P, 2, 2, P], F32, tag="petr")
                            for oi in range(2):
                                for u in range(2):
                                    nc.tensor.transpose(
                                        pst[:, oi, u, :],
                                        e_nf[:, op + oi, u * P:(u + 1) * P],
                                        identf[:])
                            nc.vector.tensor_copy(e_tp[:, op:op + 2, :, :], pst[:])
                        e_tpv = e_tp.bitcast(BF16)  # [P, no, 2, 2P]

                    expw = small_pool.tile([1, t_seg], F32, tag="expw")
                    for tci in range(ntc):
                        ts0 = tci * TC
                        osl = slice(tci * no_per_tc, (tci + 1) * no_per_tc)
                        # scores psum for this chunk
                        sc_ps = ps_sc.tile([1, TC], F32, tag="sc")
                        for ac in range(ACH):
                            pp = ps_proj.tile([P, TC], F32, tag="proj")
                            if use_xbar:
                                for hs in range(HS):
                                    nc.tensor.matmul(
                                        pp[:], w_encT[:, hs, ac * P:(ac + 1) * P],
                                        e_tx[:, osl, hs, :],
                                        start=(hs == 0), stop=(hs == HS - 1))
                            else:
                                for k, (u, par) in enumerate(
                                        [(0, 0), (0, 1), (1, 0), (1, 1)]):
                                    nc.tensor.matmul(
                                        pp[:], w2e[:, u, ac, par::2],
                                        e_tpv[:, osl, u, par::2],
                                        start=(k == 0), stop=(k == 3))
                            en = energy_pool.tile([P, TC], BF16, tag="energy")
                            nc.scalar.activation(
                                en[:], pp[:], mybir.ActivationFunctionType.Tanh,
                                bias=dec_proj[:, ac, b:b + 1], scale=1.0)
                            nc.tensor.matmul(
                                sc_ps[:], v_sb[:, ac:ac + 1], en[:],
                                start=(ac == 0), stop=(ac == ACH - 1))
                        # exp (+ partial sum)
                        nc.scalar.activation(
                            expw[0:1, ts0:ts0 + TC], sc_ps[:],
                            mybir.ActivationFunctionType.Exp,
                            accum_out=sumparts[0:1, seg * ntc + tci:seg * ntc + tci + 1])
                        # transpose expw chunk to [T_p, no_per_tc] and cast bf16
                        ew_ps = ps_tiny.tile([P, no_per_tc], F32, tag="ewT")
                        for j in range(no_per_tc):
                            nc.tensor.transpose(
                                ew_ps[:, j:j + 1],
                                expw[0:1, ts0 + j * P:ts0 + (j + 1) * P],
                                ident1[:])
                        ew_sb = small_pool.tile([P, no_per_tc], BF16, tag="ewTsb")
                        nc.vector.tensor_copy(ew_sb[:], ew_ps[:])
                        # context accumulation: 4 concurrent col-strips
                        for j in range(no_per_tc):
                            o = tci * no_per_tc + j
                            glob_o = seg * no + o
                            strip = glob_o % 4
                            nc.tensor.matmul(
                                ctx_ps[32 * strip:32 * strip + 1, :],
                                ew_sb[:, j:j + 1], e_nat[:, o, :],
                                start=(glob_o < 4), stop=(glob_o >= nseg * no - 4),
                                tile_position=(0, 32 * strip),
                                skip_group_check=True)
                    # stash unnormalized attn row in DRAM scratch
                    nc.sync.dma_start(attn_scr[b:b + 1, t0:t0 + t_seg], expw[:])
                # batch epilogue: total sum + ctx strip rows out
                nc.vector.tensor_reduce(
                    sums_all[0:1, b:b + 1], sumparts[:],
                    axis=mybir.AxisListType.X, op=mybir.AluOpType.add)
                ctx_parts = small_pool.tile([P, H], F32, tag="ctx_parts")
                for strip in range(4):
                    nc.vector.tensor_copy(
                        ctx_parts[32 * strip:32 * strip + 1, :],
                        ctx_ps[32 * strip:32 * strip + 1, :])
                for strip in range(4):
                    nc.sync.dma_start(ctx_scr4[b, strip],
                                      ctx_parts[32 * strip:32 * strip + 1, :])
                if b in (BL // 2 - 1, BL - 1):
                    h0, hn = (0, BL // 2) if b == BL // 2 - 1 else (BL // 2, BL // 2)
                    hb = BL // 2
                    s_ps = ps_tiny.tile([hb, 1], F32, tag="ewT")
                    nc.tensor.transpose(
                        s_ps[:], sums_all[0:1, h0:h0 + hb], ident1[:])
                    s_sb = small_pool.tile([hb, 1], F32, tag="sumsTsb")
                    nc.vector.tensor_copy(s_sb[:], s_ps[:])
                    recip = small_pool.tile([hb, 1], F32, tag="recip")
                    nc.vector.reciprocal(recip[:], s_sb[:])
                    attn_sb = epil_pool.tile([hb, T], F32, tag="attn_half")
                    ctx4_sb = epil_pool.tile([hb, 4, H], F32, tag="ctx4_half")
                    ctx_sb = epil_pool.tile([hb, H], F32, tag="ctx_half")
                    nc.sync.dma_start(attn_sb[:], attn_scr[h0:h0 + hb])
                    nc.sync.dma_start(ctx4_sb[:], ctx_scr4[h0:h0 + hb])
                    nc.vector.tensor_add(
                        ctx_sb[:], ctx4_sb[:, 0, :], ctx4_sb[:, 1, :])
                    nc.vector.tensor_add(ctx_sb[:], ctx_sb[:], ctx4_sb[:, 2, :])
                    nc.vector.tensor_add(ctx_sb[:], ctx_sb[:], ctx4_sb[:, 3, :])
                    nc.vector.tensor_scalar_mul(
                        attn_sb[:], attn_sb[:], recip[:, 0:1])
                    nc.vector.tensor_scalar_mul(
                        ctx_sb[:], ctx_sb[:], recip[:, 0:1])
                    nc.sync.dma_start(attn_out[h0:h0 + hb], attn_sb[:])
                    nc.sync.dma_start(ctx_out[h0:h0 + hb], ctx_sb[:])

            # ---------------- epilogue: normalize + store ----------------
            attn_sb = singles.tile([BL, T], F32)
            ctx4_sb = singles.tile([BL, 4, H], F32)
            ctx_sb = singles.tile([BL, H], F32)
            nc.sync.dma_start(attn_sb[:], attn_scr[:])
            nc.sync.dma_start(ctx4_sb[:], ctx_scr4[:])
            s_ps = ps_tiny.tile([BL, 1], F32, tag="ewT")
            nc.tensor.transpose(s_ps[:], sums_all[:], ident1[:])
            s_sb = small_pool.tile([BL, 1], F32, tag="sumsTsb")
            nc.vector.tensor_copy(s_sb[:], s_ps[:])
            recip = small_pool.tile([BL, 1], F32, tag="recip")
            nc.vector.reciprocal(recip[:], s_sb[:])
            nc.vector.tensor_add(ctx_sb[:], ctx4_sb[:, 0, :], ctx4_sb[:, 1, :])
            nc.vector.tensor_add(ctx_sb[:], ctx_sb[:], ctx4_sb[:, 2, :])
            nc.vector.tensor_add(ctx_sb[:], ctx_sb[:], ctx4_sb[:, 3, :])
            nc.vector.tensor_scalar_mul(attn_sb[:], attn_sb[:], recip[:, 0:1])
            nc.vector.tensor_scalar_mul(ctx_sb[:], ctx_sb[:], recip[:, 0:1])
            nc.sync.dma_start(attn_out[:], attn_sb[:])
            nc.sync.dma_start(ctx_out[:], ctx_sb[:])

    nc.compile()
    return nc


_cache = {}


def _get_nc(key=(N_XBAR_SEG, T_SEG)):
    if key not in _cache:
        _cache[key] = build_kernel(*key)
    return _cache[key]


def kernel(decoder_state, encoder_outputs, W_enc, W_dec, v, _trace=False):
    nc = _get_nc()
    decoder_state = np.ascontiguousarray(decoder_state, dtype=np.float32)
    encoder_outputs = np.ascontiguousarray(encoder_outputs, dtype=np.float32)
    in_maps = []
    for c in range(NCORES):
        in_maps.append({
            "enc": encoder_outputs[c * BL:(c + 1) * BL],
            "dec": decoder_state[c * BL:(c + 1) * BL],
            "w_enc": np.ascontiguousarray(W_enc, dtype=np.float32),
            "w_dec": np.ascontiguousarray(W_dec, dtype=np.float32),
            "v": np.ascontiguousarray(v, dtype=np.float32),
        })
    res = run_bass_kernel_spmd(nc, in_maps, core_ids=list(range(NCORES)),
                               trace=_trace)
    ctx = np.concatenate([r["ctx"] for r in res.results], axis=0)
    attn = np.concatenate([r["attn"] for r in res.results], axis=0)
    if _trace:
        kernel.last_results = res
    return ctx, attn


# revision 28
# speedup vs baseline: 1.0217x; 1.0217x over previous
"""Bahdanau inter-attention Trainium2 kernel (data-parallel over batch, 8 cores).

Per-core computation (BL=8 batches):
  enc_proj = enc @ W_enc.T          (bf16 PE matmul, H contracted)
  energy   = tanh(enc_proj + dec_proj)   (ACT, per-partition bias)
  scores   = energy @ v              (bf16 PE matmul, A contracted)
  expw     = exp(scores)             (ACT; no max-subtraction: |scores| < ~13)
  context  = (expw @ enc) / sum(expw)
  attn     = expw / sum(expw)

enc is loaded once from HBM with an fp32->bf16 cast in the DMA (SWDGE) and kept
in natural [T_p, H] layout for the context matmul. The projection matmul needs
H on partitions; the on-chip transpose runs (mostly) on the PE in "fp32-pair"
mode: two adjacent bf16 H-channels are moved as one fp32 element, halving the
LDWEIGHTS-bound transpose cost. The projection matmul then reads the pair
layout with stride-2 bf16 access patterns (full rate, measured). A tunable
number of segments instead use the XBAR DMA transpose (standard layout) to
offload the PE.
"""

import numpy as np

import concourse.bass as bass
import concourse.tile as tile
from concourse import bacc, mybir
from concourse.bass_utils import run_bass_kernel_spmd
from concourse.masks import make_identity

F32 = mybir.dt.float32
BF16 = mybir.dt.bfloat16

B, T, H, A = 64, 4096, 512, 256
NCORES = 8
BL = B // NCORES          # 8 batches per core
P = 128

# ---- tunables ----
T_SEG = 2048              # T tile per DMA load
TC = 512                  # matmul moving-dim chunk
N_XBAR_SEG = 0            # of the 16 (b, seg) units, how many transpose via XBAR

ACH = A // P              # A chunks of 128
HS = H // P               # H subtiles (contraction)


def build_kernel(n_xbar_seg=N_XBAR_SEG, t_seg=T_SEG):
    nseg = T // t_seg
    ntc = t_seg // TC
    no = t_seg // P
    no_per_tc = TC // P
    gtot = BL * nseg
    if n_xbar_seg > 0:
        xbar_set = set(int(round(i * gtot / n_xbar_seg)) % gtot
                       for i in range(n_xbar_seg))
    else:
        xbar_set = set()

    nc = bacc.Bacc("TRN2", target_bir_lowering=False, debug=False)

    enc = nc.dram_tensor("enc", [BL, T, H], F32, kind="ExternalInput")
    dec = nc.dram_tensor("dec", [BL, H], F32, kind="ExternalInput")
    w_enc = nc.dram_tensor("w_enc", [A, H], F32, kind="ExternalInput")
    w_dec = nc.dram_tensor("w_dec", [A, H], F32, kind="ExternalInput")
    v_in = nc.dram_tensor("v", [1, A], F32, kind="ExternalInput")
    ctx_out = nc.dram_tensor("ctx", [BL, H], F32, kind="ExternalOutput")
    attn_out = nc.dram_tensor("attn", [BL, T], F32, kind="ExternalOutput")

    with tile.TileContext(nc) as tc:
        with tc.tile_pool(name="singles", bufs=1) as singles, \
             tc.tile_pool(name="enc_nat", bufs=3) as enc_nat_pool, \
             tc.tile_pool(name="enc_t", bufs=2) as enc_t_pool, \
             tc.tile_pool(name="energy", bufs=4) as energy_pool, \
             tc.tile_pool(name="small", bufs=3) as small_pool, \
             tc.tile_pool(name="epil", bufs=1) as epil_pool, \
             tc.tile_pool(name="dram", bufs=1, space="DRAM") as dram_pool, \
             tc.tile_pool(name="ps_proj", bufs=3, space="PSUM") as ps_proj, \
             tc.tile_pool(name="ps_sc", bufs=1, space="PSUM") as ps_sc, \
             tc.tile_pool(name="ps_tiny", bufs=1, space="PSUM") as ps_tiny, \
             tc.tile_pool(name="ps_petr", bufs=2, space="PSUM") as ps_petr, \
             tc.tile_pool(name="ps_ctx", bufs=1, space="PSUM") as ps_ctx:

            # ---------------- prep: identities ----------------
            ident_bf = singles.tile([P, P], BF16)
            make_identity(nc, ident_bf[:])
            identf = singles.tile([P, P], F32)
            make_identity(nc, identf[:])
            ident1 = singles.tile([1, 1], F32)
            nc.vector.memset(ident1[:], 1.0)

            # -------- prep: weights --------
            # standard transposed [H_p, hs, A] bf16 (for XBAR segs)
            # pair layout [j, u, ac, a] fp32-pairs: elem = (W[a, 256u+2j], W[a, 256u+2j+1])
            def load_w(w_dram, name, want_std):
                w_sb = small_pool.tile([P, ACH, H], BF16, tag="w_load")
                nc.gpsimd.dma_start(
                    w_sb[:], w_dram.rearrange("(s p) h -> p s h", p=P))
                wT = None
                if want_std:
                    wT = singles.tile([P, HS, A], BF16, tag=f"wT_{name}")
                    for asub in range(ACH):
                        for hs in range(HS):
                            pst = ps_proj.tile([P, P], BF16, tag="proj")
                            nc.tensor.transpose(
                                pst[:], w_sb[:, asub, hs * P:(hs + 1) * P],
                                ident_bf[:])
                            nc.vector.tensor_copy(
                                wT[:, hs, asub * P:(asub + 1) * P], pst[:])
                w2 = singles.tile([P, 2, ACH, P], F32, tag=f"w2_{name}")
                w_f32 = w_sb.bitcast(F32)  # [P, ACH, 256]
                for u in range(2):
                    for asub in range(ACH):
                        pst = ps_proj.tile([P, P], F32, tag="proj")
                        nc.tensor.transpose(
                            pst[:], w_f32[:, asub, u * P:(u + 1) * P], identf[:])
                        nc.vector.tensor_copy(w2[:, u, asub, :], pst[:])
                return wT, w2.bitcast(BF16)  # w2 view [P, 2, ACH, 2P] (a,parity interleaved)

            w_encT, w2e = load_w(w_enc, "enc", n_xbar_seg > 0)
            _, w2d = load_w(w_dec, "dec", False)

            # v as [A_p, ACH] bf16: v_sb[p, c] = v[0, c*128+p]
            v_sb = singles.tile([P, ACH], BF16)
            nc.gpsimd.dma_start(v_sb[:], v_in[0].rearrange("(c p) -> p c", p=P))

            # -------- prep: dec_proj [A_p, ACH, BL] fp32 --------
            dec_sb = small_pool.tile([BL, H], BF16, tag="dec_load")
            nc.gpsimd.dma_start(dec_sb[:], dec[:])
            dec_f32 = dec_sb.bitcast(F32)  # [BL, 256]
            decT2 = singles.tile([P, 2, BL], F32)
            for u in range(2):
                pst = ps_proj.tile([P, BL], F32, tag="proj")
                nc.tensor.transpose(
                    pst[:], dec_f32[:, u * P:(u + 1) * P], identf[:BL, :BL])
                nc.vector.tensor_copy(decT2[:, u, :], pst[:])
            decT2v = decT2.bitcast(BF16)  # [P, 2, 2*BL]
            dec_proj = singles.tile([P, ACH, BL], F32)
            for ac in range(ACH):
                psd = ps_proj.tile([P, BL], F32, tag="proj")
                for k, (u, par) in enumerate([(0, 0), (0, 1), (1, 0), (1, 1)]):
                    nc.tensor.matmul(
                        psd[:], w2d[:, u, ac, par::2], decT2v[:, u, par::2],
                        start=(k == 0), stop=(k == 3))
                nc.vector.tensor_copy(dec_proj[:, ac, :], psd[:])

            # DRAM scratch for unnormalized per-batch rows (engine APs cannot
            # start at partition b; DMA can address any DRAM row).
            attn_scr = dram_pool.tile([BL, T], F32)
            ctx_scr4 = dram_pool.tile([BL, 4, H], F32)
            sums_all = singles.tile([1, BL], F32)

            # ---------------- main loop ----------------
            for b in range(BL):
                ctx_ps = ps_ctx.tile([P, H], F32, tag="ctx")
                sumparts = small_pool.tile([1, nseg * ntc], F32, tag="sumparts")
                for seg in range(nseg):
                    g = b * nseg + seg
                    use_xbar = g in xbar_set
                    t0 = seg * t_seg
                    # load natural tile [128, no, H] bf16 (cast in DMA)
                    e_nat = enc_nat_pool.tile([P, no, H], BF16, tag="enat")
                    nc.gpsimd.dma_start(
                        e_nat[:],
                        enc[b, t0:t0 + t_seg, :].rearrange("(o p) h -> p o h", p=P))

                    if use_xbar:
                        # one batched XBAR transpose -> [H_p, o, hs, 128] bf16,
                        # alternating HWDGE rings (SP / ACT)
                        e_tx = enc_t_pool.tile([P, no, HS, P], BF16, tag="etr_x")
                        ring = nc.sync if g % 2 == 0 else nc.scalar
                        ring.dma_start(e_tx[:], e_nat[:], transpose=True)
                    else:
                        # PE fp32-pair transposes -> [j, o, u, t] fp32-pairs
                        e_tp = enc_t_pool.tile([P, no, 2, P], F32, tag="etr_p")
                        e_nf = e_nat.bitcast(F32)  # [P, no, 256]
                        for op in range(0, no, 2):
                            pst = ps_petr.tile([P, 2, 2, P], F32, tag="petr")
                            for oi in range(2):
                                for u in range(2):
                                    nc.tensor.transpose(
                                        pst[:, oi, u, :],
                                        e_nf[:, op + oi, u * P:(u + 1) * P],
                                        identf[:])
                            nc.vector.tensor_copy(e_tp[:, op:op + 2, :, :], pst[:])
                        e_tpv = e_tp.bitcast(BF16)  # [P, no, 2, 2P]

                    expw = small_pool.tile([1, t_seg], F32, tag="expw")
                    for tci in range(ntc):
                        ts0 = tci * TC
                        osl = slice(tci * no_per_tc, (tci + 1) * no_per_tc)
                        # scores psum for this chunk
                        sc_ps = ps_sc.tile([1, TC], F32, tag="sc")
                        for ac in range(ACH):
                            pp = ps_proj.tile([P, TC], F32, tag="proj")
                            if use_xbar:
                                for hs in range(HS):
                                    nc.tensor.matmul(
                                        pp[:], w_encT[:, hs, ac * P:(ac + 1) * P],
                                        e_tx[:, osl, hs, :],
                                        start=(hs == 0), stop=(hs == HS - 1))
                            else:
                                for k, (u, par) in enumerate(
                                        [(0, 0), (0, 1), (1, 0), (1, 1)]):
                                    nc.tensor.matmul(
                                        pp[:], w2e[:, u, ac, par::2],
                                        e_tpv[:, osl, u, par::2],
                                        start=(k == 0), stop=(k == 3))
                            en = energy_pool.tile([P, TC], BF16, tag="energy")
                            nc.scalar.activation(
                                en[:], pp[:], mybir.ActivationFunctionType.Tanh,
                                bias=dec_proj[:, ac, b:b + 1], scale=1.0)
                            nc.tensor.matmul(
                                sc_ps[:], v_sb[:, ac:ac + 1], en[:],
                                start=(ac == 0), stop=(ac == ACH - 1))
                        # exp (+ partial sum)
                        nc.scalar.activation(
                            expw[0:1, ts0:ts0 + TC], sc_ps[:],
                            mybir.ActivationFunctionType.Exp,
                            accum_out=sumparts[0:1, seg * ntc + tci:seg * ntc + tci + 1])
                        # transpose expw chunk to [T_p, no_per_tc] and cast bf16
                        ew_ps = ps_tiny.tile([P, no_per_tc], F32, tag="ewT")
                        for j in range(no_per_tc):
                            nc.tensor.transpose(
                                ew_ps[:, j:j + 1],
                                expw[0:1, ts0 + j * P:ts0 + (j + 1) * P],
                                ident1[:])
                        ew_sb = small_pool.tile([P, no_per_tc], BF16, tag="ewTsb")
                        nc.vector.tensor_copy(ew_sb[:], ew_ps[:])
                        # context accumulation: 4 concurrent col-strips
                        for j in range(no_per_tc):
                            o = tci * no_per_tc + j
                            glob_o = seg * no + o
                            strip = glob_o % 4
                            nc.tensor.matmul(
                                ctx_ps[32 * strip:32 * strip + 1, :],
                                ew_sb[:, j:j + 1], e_nat[:, o, :],
                                start=(glob_o < 4), stop=(glob_o >= nseg * no - 4),
                                tile_position=(0, 32 * strip),
                                skip_group_check=True)
                    # stash unnormalized attn row in DRAM scratch
                    nc.sync.dma_start(attn_scr[b:b + 1, t0:t0 + t_seg], expw[:])
                # batch epilogue: total sum + ctx strip rows out
                nc.vector.tensor_reduce(
                    sums_all[0:1, b:b + 1], sumparts[:],
                    axis=mybir.AxisListType.X, op=mybir.AluOpType.add)
                ctx_parts = small_pool.tile([P, H], F32, tag="ctx_parts")
                for strip in range(4):
                    nc.vector.tensor_copy(
                        ctx_parts[32 * strip:32 * strip + 1, :],
                        ctx_ps[32 * strip:32 * strip + 1, :])
                for strip in range(4):
                    nc.sync.dma_start(ctx_scr4[b, strip],
                                      ctx_parts[32 * strip:32 * strip + 1, :])

            # ---------------- epilogue: normalize + store ----------------
            attn_sb = singles.tile([BL, T], F32)
            ctx4_sb = singles.tile([BL, 4, H], F32)
            ctx_sb = singles.tile([BL, H], F32)
            nc.sync.dma_start(attn_sb[:], attn_scr[:])
            nc.sync.dma_start(ctx4_sb[:], ctx_scr4[:])
            s_ps = ps_tiny.tile([BL, 1], F32, tag="ewT")
            nc.tensor.transpose(s_ps[:], sums_all[:], ident1[:])
            s_sb = small_pool.tile([BL, 1], F32, tag="sumsTsb")
            nc.vector.tensor_copy(s_sb[:], s_ps[:])
            recip = small_pool.tile([BL, 1], F32, tag="recip")
            nc.vector.reciprocal(recip[:], s_sb[:])
            nc.vector.tensor_add(ctx_sb[:], ctx4_sb[:, 0, :], ctx4_sb[:, 1, :])
            nc.vector.tensor_add(ctx_sb[:], ctx_sb[:], ctx4_sb[:, 2, :])
            nc.vector.tensor_add(ctx_sb[:], ctx_sb[:], ctx4_sb[:, 3, :])
            nc.vector.tensor_scalar_mul(attn_sb[:], attn_sb[:], recip[:, 0:1])
            nc.vector.tensor_scalar_mul(ctx_sb[:], ctx_sb[:], recip[:, 0:1])
            nc.sync.dma_start(attn_out[:], attn_sb[:])
            nc.sync.dma_start(ctx_out[:], ctx_sb[:])

    nc.compile()
    return nc


_cache = {}


def _get_nc(key=(N_XBAR_SEG, T_SEG)):
    if key not in _cache:
        _cache[key] = build_kernel(*key)
    return _cache[key]


def kernel(decoder_state, encoder_outputs, W_enc, W_dec, v, _trace=False):
    nc = _get_nc()
    decoder_state = np.ascontiguousarray(decoder_state, dtype=np.float32)
    encoder_outputs = np.ascontiguousarray(encoder_outputs, dtype=np.float32)
    in_maps = []
    for c in range(NCORES):
        in_maps.append({
            "enc": encoder_outputs[c * BL:(c + 1) * BL],
            "dec": decoder_state[c * BL:(c + 1) * BL],
            "w_enc": np.ascontiguousarray(W_enc, dtype=np.float32),
            "w_dec": np.ascontiguousarray(W_dec, dtype=np.float32),
            "v": np.ascontiguousarray(v, dtype=np.float32),
        })
    res = run_bass_kernel_spmd(nc, in_maps, core_ids=list(range(NCORES)),
                               trace=_trace)
    ctx = np.concatenate([r["ctx"] for r in res.results], axis=0)
    attn = np.concatenate([r["attn"] for r in res.results], axis=0)
    if _trace:
        kernel.last_results = res
    return ctx, attn


# revision 29
# speedup vs baseline: 1.0257x; 1.0039x over previous
"""Bahdanau inter-attention Trainium2 kernel (data-parallel over batch, 8 cores).

Per-core computation (BL=8 batches):
  enc_proj = enc @ W_enc.T          (bf16 PE matmul, H contracted)
  energy   = tanh(enc_proj + dec_proj)   (ACT, per-partition bias)
  scores   = energy @ v              (bf16 PE matmul, A contracted)
  expw     = exp(scores)             (ACT; no max-subtraction: |scores| < ~13)
  context  = (expw @ enc) / sum(expw)
  attn     = expw / sum(expw)

enc is loaded once from HBM with an fp32->bf16 cast in the DMA (SWDGE) and kept
in natural [T_p, H] layout for the context matmul. The projection matmul needs
H on partitions; the on-chip transpose runs (mostly) on the PE in "fp32-pair"
mode: two adjacent bf16 H-channels are moved as one fp32 element, halving the
LDWEIGHTS-bound transpose cost. The projection matmul then reads the pair
layout with stride-2 bf16 access patterns (full rate, measured). A tunable
number of segments instead use the XBAR DMA transpose (standard layout) to
offload the PE.
"""

import numpy as np

import concourse.bass as bass
import concourse.tile as tile
from concourse import bacc, mybir
from concourse.bass_utils import run_bass_kernel_spmd
from concourse.masks import make_identity

F32 = mybir.dt.float32
BF16 = mybir.dt.bfloat16

B, T, H, A = 64, 4096, 512, 256
NCORES = 8
BL = B // NCORES          # 8 batches per core
P = 128

# ---- tunables ----
T_SEG = 2048              # T tile per DMA load
TC = 512                  # matmul moving-dim chunk
N_XBAR_SEG = 0            # of the 16 (b, seg) units, how many transpose via XBAR

ACH = A // P              # A chunks of 128
HS = H // P               # H subtiles (contraction)


def build_kernel(n_xbar_seg=N_XBAR_SEG, t_seg=T_SEG):
    nseg = T // t_seg
    ntc = t_seg // TC
    no = t_seg // P
    no_per_tc = TC // P
    gtot = BL * nseg
    if n_xbar_seg > 0:
        xbar_set = set(int(round(i * gtot / n_xbar_seg)) % gtot
                       for i in range(n_xbar_seg))
    else:
        xbar_set = set()

    nc = bacc.Bacc("TRN2", target_bir_lowering=False, debug=False)

    enc = nc.dram_tensor("enc", [BL, T, H], F32, kind="ExternalInput")
    dec = nc.dram_tensor("dec", [BL, H], F32, kind="ExternalInput")
    w_enc = nc.dram_tensor("w_enc", [A, H], F32, kind="ExternalInput")
    w_dec = nc.dram_tensor("w_dec", [A, H], F32, kind="ExternalInput")
    v_in = nc.dram_tensor("v", [1, A], F32, kind="ExternalInput")
    ctx_out = nc.dram_tensor("ctx", [BL, H], F32, kind="ExternalOutput")
    attn_out = nc.dram_tensor("attn", [BL, T], F32, kind="ExternalOutput")

    with tile.TileContext(nc) as tc:
        with tc.tile_pool(name="singles", bufs=1) as singles, \
             tc.tile_pool(name="enc_nat", bufs=3) as enc_nat_pool, \
             tc.tile_pool(name="enc_t", bufs=2) as enc_t_pool, \
             tc.tile_pool(name="energy", bufs=4) as energy_pool, \
             tc.tile_pool(name="small", bufs=3) as small_pool, \
             tc.tile_pool(name="epil", bufs=1) as epil_pool, \
             tc.tile_pool(name="dram", bufs=1, space="DRAM") as dram_pool, \
             tc.tile_pool(name="ps_proj", bufs=3, space="PSUM") as ps_proj, \
             tc.tile_pool(name="ps_sc", bufs=1, space="PSUM") as ps_sc, \
             tc.tile_pool(name="ps_tiny", bufs=1, space="PSUM") as ps_tiny, \
             tc.tile_pool(name="ps_petr", bufs=2, space="PSUM") as ps_petr, \
             tc.tile_pool(name="ps_ctx", bufs=1, space="PSUM") as ps_ctx:

            # ---------------- prep: identities ----------------
            ident_bf = singles.tile([P, P], BF16)
            make_identity(nc, ident_bf[:])
            identf = singles.tile([P, P], F32)
            make_identity(nc, identf[:])
            ident1 = singles.tile([1, 1], F32)
            nc.vector.memset(ident1[:], 1.0)

            # -------- prep: weights --------
            # standard transposed [H_p, hs, A] bf16 (for XBAR segs)
            # pair layout [j, u, ac, a] fp32-pairs: elem = (W[a, 256u+2j], W[a, 256u+2j+1])
            def load_w(w_dram, name, want_std):
                w_sb = small_pool.tile([P, ACH, H], BF16, tag="w_load")
                nc.gpsimd.dma_start(
                    w_sb[:], w_dram.rearrange("(s p) h -> p s h", p=P))
                wT = None
                if want_std:
                    wT = singles.tile([P, HS, A], BF16, tag=f"wT_{name}")
                    for asub in range(ACH):
                        for hs in range(HS):
                            pst = ps_proj.tile([P, P], BF16, tag="proj")
                            nc.tensor.transpose(
                                pst[:], w_sb[:, asub, hs * P:(hs + 1) * P],
                                ident_bf[:])
                            nc.vector.tensor_copy(
                                wT[:, hs, asub * P:(asub + 1) * P], pst[:])
                w2 = singles.tile([P, 2, ACH, P], F32, tag=f"w2_{name}")
                w_f32 = w_sb.bitcast(F32)  # [P, ACH, 256]
                for u in range(2):
                    for asub in range(ACH):
                        pst = ps_proj.tile([P, P], F32, tag="proj")
                        nc.tensor.transpose(
                            pst[:], w_f32[:, asub, u * P:(u + 1) * P], identf[:])
                        nc.vector.tensor_copy(w2[:, u, asub, :], pst[:])
                return wT, w2.bitcast(BF16)  # w2 view [P, 2, ACH, 2P] (a,parity interleaved)

            w_encT, w2e = load_w(w_enc, "enc", n_xbar_seg > 0)
            _, w2d = load_w(w_dec, "dec", False)

            # v as [A_p, ACH] bf16: v_sb[p, c] = v[0, c*128+p]
            v_sb = singles.tile([P, ACH], BF16)
            nc.gpsimd.dma_start(v_sb[:], v_in[0].rearrange("(c p) -> p c", p=P))

            # -------- prep: dec_proj [A_p, ACH, BL] fp32 --------
            dec_sb = small_pool.tile([BL, H], BF16, tag="dec_load")
            nc.gpsimd.dma_start(dec_sb[:], dec[:])
            dec_f32 = dec_sb.bitcast(F32)  # [BL, 256]
            decT2 = singles.tile([P, 2, BL], F32)
            for u in range(2):
                pst = ps_proj.tile([P, BL], F32, tag="proj")
                nc.tensor.transpose(
                    pst[:], dec_f32[:, u * P:(u + 1) * P], identf[:BL, :BL])
                nc.vector.tensor_copy(decT2[:, u, :], pst[:])
            decT2v = decT2.bitcast(BF16)  # [P, 2, 2*BL]
            dec_proj = singles.tile([P, ACH, BL], F32)
            for ac in range(ACH):
                psd = ps_proj.tile([P, BL], F32, tag="proj")
                for k, (u, par) in enumerate([(0, 0), (0, 1), (1, 0), (1, 1)]):
                    nc.tensor.matmul(
                        psd[:], w2d[:, u, ac, par::2], decT2v[:, u, par::2],
                        start=(k == 0), stop=(k == 3))
                nc.vector.tensor_copy(dec_proj[:, ac, :], psd[:])

            # DRAM scratch for unnormalized per-batch rows (engine APs cannot
            # start at partition b; DMA can address any DRAM row).
            attn_scr = dram_pool.tile([BL, T], F32)
            ctx_scr4 = dram_pool.tile([BL, 4, H], F32)
            sums_all = singles.tile([1, BL], F32)

            # ---------------- main loop ----------------
            for b in range(BL):
                ctx_ps = ps_ctx.tile([P, H], F32, tag="ctx")
                sumparts = small_pool.tile([1, nseg * ntc], F32, tag="sumparts")
                for seg in range(nseg):
                    g = b * nseg + seg
                    use_xbar = g in xbar_set
                    t0 = seg * t_seg
                    # load natural tile [128, no, H] bf16 (cast in DMA)
                    e_nat = enc_nat_pool.tile([P, no, H], BF16, tag="enat")
                    nc.gpsimd.dma_start(
                        e_nat[:],
                        enc[b, t0:t0 + t_seg, :].rearrange("(o p) h -> p o h", p=P))

                    if use_xbar:
                        # one batched XBAR transpose -> [H_p, o, hs, 128] bf16,
                        # alternating HWDGE rings (SP / ACT)
                        e_tx = enc_t_pool.tile([P, no, HS, P], BF16, tag="etr_x")
                        ring = nc.sync if g % 2 == 0 else nc.scalar
                        ring.dma_start(e_tx[:], e_nat[:], transpose=True)
                    else:
                        # PE fp32-pair transposes -> [j, o, u, t] fp32-pairs
                        e_tp = enc_t_pool.tile([P, no, 2, P], F32, tag="etr_p")
                        e_nf = e_nat.bitcast(F32)  # [P, no, 256]
                        for op in range(0, no, 2):
                            pst = ps_petr.tile([P, 2, 2, P], F32, tag="petr")
                            for oi in range(2):
                                for u in range(2):
                                    nc.tensor.transpose(
                                        pst[:, oi, u, :],
                                        e_nf[:, op + oi, u * P:(u + 1) * P],
                                        identf[:])
                            if (op // 2) % 2 == 0:
                                nc.vector.tensor_copy(
                                    e_tp[:, op:op + 2, :, :], pst[:])
                            else:
                                nc.scalar.copy(e_tp[:, op:op + 2, :, :], pst[:])
                        e_tpv = e_tp.bitcast(BF16)  # [P, no, 2, 2P]

                    expw = small_pool.tile([1, t_seg], F32, tag="expw")
                    for tci in range(ntc):
                        ts0 = tci * TC
                        osl = slice(tci * no_per_tc, (tci + 1) * no_per_tc)
                        # scores psum for this chunk
                        sc_ps = ps_sc.tile([1, TC], F32, tag="sc")
                        for ac in range(ACH):
                            pp = ps_proj.tile([P, TC], F32, tag="proj")
                            if use_xbar:
                                for hs in range(HS):
                                    nc.tensor.matmul(
                                        pp[:], w_encT[:, hs, ac * P:(ac + 1) * P],
                                        e_tx[:, osl, hs, :],
                                        start=(hs == 0), stop=(hs == HS - 1))
                            else:
                                for k, (u, par) in enumerate(
                                        [(0, 0), (0, 1), (1, 0), (1, 1)]):
                                    nc.tensor.matmul(
                                        pp[:], w2e[:, u, ac, par::2],
                                        e_tpv[:, osl, u, par::2],
                                        start=(k == 0), stop=(k == 3))
                            en = energy_pool.tile([P, TC], BF16, tag="energy")
                            nc.scalar.activation(
                                en[:], pp[:], mybir.ActivationFunctionType.Tanh,
                                bias=dec_proj[:, ac, b:b + 1], scale=1.0)
                            nc.tensor.matmul(
                                sc_ps[:], v_sb[:, ac:ac + 1], en[:],
                                start=(ac == 0), stop=(ac == ACH - 1))
                        # exp (+ partial sum)
                        nc.scalar.activation(
                            expw[0:1, ts0:ts0 + TC], sc_ps[:],
                            mybir.ActivationFunctionType.Exp,
                            accum_out=sumparts[0:1, seg * ntc + tci:seg * ntc + tci + 1])
                        # transpose expw chunk to [T_p, no_per_tc] and cast bf16
                        ew_ps = ps_tiny.tile([P, no_per_tc], F32, tag="ewT")
                        for j in range(no_per_tc):
                            nc.tensor.transpose(
                                ew_ps[:, j:j + 1],
                                expw[0:1, ts0 + j * P:ts0 + (j + 1) * P],
                                ident1[:])
                        ew_sb = small_pool.tile([P, no_per_tc], BF16, tag="ewTsb")
                        nc.vector.tensor_copy(ew_sb[:], ew_ps[:])
                        # context accumulation: 4 concurrent col-strips
                        for j in range(no_per_tc):
                            o = tci * no_per_tc + j
                            glob_o = seg * no + o
                            strip = glob_o % 4
                            nc.tensor.matmul(
                                ctx_ps[32 * strip:32 * strip + 1, :],
                                ew_sb[:, j:j + 1], e_nat[:, o, :],
                                start=(glob_o < 4), stop=(glob_o >= nseg * no - 4),
                                tile_position=(0, 32 * strip),
                                skip_group_check=True)
                    # stash unnormalized attn row in DRAM scratch
                    nc.sync.dma_start(attn_scr[b:b + 1, t0:t0 + t_seg], expw[:])
                # batch epilogue: total sum + ctx strip rows out
                nc.vector.tensor_reduce(
                    sums_all[0:1, b:b + 1], sumparts[:],
                    axis=mybir.AxisListType.X, op=mybir.AluOpType.add)
                ctx_parts = small_pool.tile([P, H], F32, tag="ctx_parts")
                for strip in range(4):
                    nc.vector.tensor_copy(
                        ctx_parts[32 * strip:32 * strip + 1, :],
                        ctx_ps[32 * strip:32 * strip + 1, :])
                for strip in range(4):
                    nc.sync.dma_start(ctx_scr4[b, strip],
                                      ctx_parts[32 * strip:32 * strip + 1, :])

            # ---------------- epilogue: normalize + store ----------------
            attn_sb = singles.tile([BL, T], F32)
            ctx4_sb = singles.tile([BL, 4, H], F32)
            ctx_sb = singles.tile([BL, H], F32)
            nc.sync.dma_start(attn_sb[:], attn_scr[:])
            nc.sync.dma_start(ctx4_sb[:], ctx_scr4[:])
            s_ps = ps_tiny.tile([BL, 1], F32, tag="ewT")
            nc.tensor.transpose(s_ps[:], sums_all[:], ident1[:])
            s_sb = small_pool.tile([BL, 1], F32, tag="sumsTsb")
            nc.vector.tensor_copy(s_sb[:], s_ps[:])
            recip = small_pool.tile([BL, 1], F32, tag="recip")
            nc.vector.reciprocal(recip[:], s_sb[:])
            nc.vector.tensor_add(ctx_sb[:], ctx4_sb[:, 0, :], ctx4_sb[:, 1, :])
            nc.vector.tensor_add(ctx_sb[:], ctx_sb[:], ctx4_sb[:, 2, :])
            nc.vector.tensor_add(ctx_sb[:], ctx_sb[:], ctx4_sb[:, 3, :])
            nc.vector.tensor_scalar_mul(attn_sb[:], attn_sb[:], recip[:, 0:1])
            nc.vector.tensor_scalar_mul(ctx_sb[:], ctx_sb[:], recip[:, 0:1])
            nc.sync.dma_start(attn_out[:], attn_sb[:])
            nc.sync.dma_start(ctx_out[:], ctx_sb[:])

    nc.compile()
    return nc


_cache = {}


def _get_nc(key=(N_XBAR_SEG, T_SEG)):
    if key not in _cache:
        _cache[key] = build_kernel(*key)
    return _cache[key]


def kernel(decoder_state, encoder_outputs, W_enc, W_dec, v, _trace=False):
    nc = _get_nc()
    decoder_state = np.ascontiguousarray(decoder_state, dtype=np.float32)
    encoder_outputs = np.ascontiguousarray(encoder_outputs, dtype=np.float32)
    in_maps = []
    for c in range(NCORES):
        in_maps.append({
            "enc": encoder_outputs[c * BL:(c + 1) * BL],
            "dec": decoder_state[c * BL:(c + 1) * BL],
            "w_enc": np.ascontiguousarray(W_enc, dtype=np.float32),
            "w_dec": np.ascontiguousarray(W_dec, dtype=np.float32),
            "v": np.ascontiguousarray(v, dtype=np.float32),
        })
    res = run_bass_kernel_spmd(nc, in_maps, core_ids=list(range(NCORES)),
                               trace=_trace)
    ctx = np.concatenate([r["ctx"] for r in res.results], axis=0)
    attn = np.concatenate([r["attn"] for r in res.results], axis=0)
    if _trace:
        kernel.last_results = res
    return ctx, attn
